# revision 2
# baseline (speedup 1.0000x reference)
"""Permutation scatter: out[perm[i]] = inputs[i]  (B=131072, D=512, f32).

Since perm is a permutation, out[j] = inputs[inv_perm[j]] -- a pure row
gather.  Strategy: shard the OUTPUT rows across the 8 cores and replicate
the full input to every core.  Core k owns output rows [k*R, (k+1)*R) and
gathers its 16384 rows (2 KiB each) from its local replica with indirect
DMAs, then writes its output shard contiguously.  No collectives; per-core
HBM traffic is the minimum possible (32 MiB read + 32 MiB write).  The
host only computes the inverse permutation (index math); all payload
movement happens on-device.

HW contract for indirect DMA (probed): one index per partition, dest AP
[128, D].  So each gather chunk covers 128 output rows; 128 chunks/core.
The per-core index tensor is passed pre-transposed (idxT[p, c] =
inv_k[c*128 + p]) so a single contiguous [128, 128] SBUF tile holds one
chunk's indices per column.
"""

import numpy as np

B = 131072
D = 512
N_CORES = 8
R = B // N_CORES  # 16384 output rows per core
P = 128
NCH = R // P  # 128 chunks per core

DATA_BUFS = 12

_cached = None


def _build_nc(data_bufs=DATA_BUFS):
    import concourse.bacc as bacc
    import concourse.bass as bass
    import concourse.mybir as mybir
    import concourse.tile as tile

    nc = bacc.Bacc(
        "TRN2",
        target_bir_lowering=False,
        debug=False,
        num_devices=N_CORES,
    )

    x = nc.dram_tensor("x", [B, D], mybir.dt.float32, kind="ExternalInput")
    # idxT[p, c] = source row for output row c*128 + p (core-local)
    idxT = nc.dram_tensor("idxT", [P, NCH], mybir.dt.int32, kind="ExternalInput")
    y = nc.dram_tensor("y", [R, D], mybir.dt.float32, kind="ExternalOutput")

    y_r = y[:].rearrange("(c p) d -> c p d", p=P)

    with tile.TileContext(nc) as tc:
        with (
            tc.tile_pool(name="idxp", bufs=1) as ipool,
            tc.tile_pool(name="data", bufs=data_bufs) as dpool,
        ):
            it = ipool.tile([P, NCH], mybir.dt.int32)
            nc.sync.dma_start(out=it[:], in_=idxT[:])
            for c in range(NCH):
                dtile = dpool.tile([P, D], mybir.dt.float32)
                nc.gpsimd.indirect_dma_start(
                    out=dtile[:],
                    out_offset=None,
                    in_=x[:],
                    in_offset=bass.IndirectOffsetOnAxis(ap=it[:, c : c + 1], axis=0),
                )
                nc.sync.dma_start(out=y_r[c], in_=dtile[:])

    nc.compile()
    return nc


def _get_nc():
    global _cached
    if _cached is None:
        _cached = _build_nc()
    return _cached


def _make_in_maps(inputs, perm):
    x = np.ascontiguousarray(np.asarray(inputs, dtype=np.float32))
    p = np.asarray(perm).astype(np.int64)
    inv = np.empty(B, dtype=np.int32)
    inv[p] = np.arange(B, dtype=np.int32)
    maps = []
    for k in range(N_CORES):
        sl = inv[k * R : (k + 1) * R].reshape(NCH, P)  # [c, p]
        maps.append({"x": x, "idxT": np.ascontiguousarray(sl.T)})
    return maps


def kernel(**kw):
    from concourse.bass_utils import run_bass_kernel_spmd

    nc = _get_nc()
    in_maps = _make_in_maps(kw["inputs"], kw["perm"])
    res = run_bass_kernel_spmd(nc, in_maps, core_ids=list(range(N_CORES)))
    return np.concatenate([res.results[k]["y"] for k in range(N_CORES)], axis=0)


def run_traced(inputs, perm, **trace_kw):
    """test.py helper: same as kernel() but returns (out, BassKernelResults)."""
    from concourse.bass_utils import run_bass_kernel_spmd

    nc = _get_nc()
    in_maps = _make_in_maps(inputs, perm)
    res = run_bass_kernel_spmd(
        nc, in_maps, core_ids=list(range(N_CORES)), trace=True, **trace_kw
    )
    out = np.concatenate([res.results[k]["y"] for k in range(N_CORES)], axis=0)
    return out, res
